# revision 1
# baseline (speedup 1.0000x reference)
"""Trainium2 Bass kernel: GarmentPersonCrossAttention (B=4, N=2048, M=1024,
DQ=1024, DC=768, H=16, DH=64), distributed over 8 NeuronCores.

Sharding: core i handles batch i//2 and person-row half i%2 (1024 rows).
Everything is local per core (garment-side LN + K/V projections are
recomputed by both cores of a batch pair) -- no collectives.

Host-side algebraic folds (exact linear algebra, numpy):
  - LN affine (gamma, beta) folded into Wq/Wk/Wv plus bias rows.
  - softmax scale DH**-0.5 folded into Wq (and its bias).
  - concat([residual, att]) @ Wf + bf
        = residual @ Wf[:DQ] + att @ (Wo @ Wf[DQ:]) + (bo @ Wf[DQ:] + bf)
    so Wo and the bottom half of Wf collapse into one matrix WoF.

Device pipeline per core (bf16 matmuls, fp32 PSUM accumulation):
  A: load x_p/x_g row-major (bf16), LayerNorm stats via bn_stats, apply
     (x-mu)*rstd, store z to DRAM scratch and reload feature-major via DMA
     transpose (zpT, zgT). The residual xT is DMA-transposed straight from
     the input in phase D.
  B: qT = Wq'.T @ zpT, kT = Wk'.T @ zgT (biases added on ACT during PSUM
     evacuation); v = zgT.T @ Wv' row-major with a ones column appended per
     head (the 65th row of the att matmul then yields softmax denominators
     for free).
  C: per (head, n-chunk): scoresT[m,n] = kT.T @ qT, exp on ACT
     (PSUM->SBUF, bf16), attT[65,n] = v_aug.T @ exp accumulated over m.
     Softmax normalization: reciprocal of row 64, broadcast across 64
     partitions via a DRAM bounce, multiplied in during PSUM evacuation.
  D: out[n,dq] = xT.T @ Wf_top + attT.T @ WoF + bias row, evacuated with a
     fused tensor_add (fp32 out).
"""

import os
import sys

import numpy as np

for _p in ("/opt/trn_rl_repo",):
    if _p not in sys.path and os.path.isdir(_p):
        sys.path.append(_p)

import ml_dtypes

# Problem constants (hardcoded per contest rules).
B, N, M = 4, 2048, 1024
DQ, DC = 1024, 768
H, DH = 16, 64
INNER = H * DH
SCALE = DH ** -0.5
EPS = 1e-5
NCORES = 8
NPC = N // 2          # person rows per core
P = 128               # partitions
NT = NPC // P         # 8 person row tiles per core
MT = M // P           # 8 garment row tiles
KQ = DQ // P          # 8 contraction tiles for person features
KC = DC // P          # 6 contraction tiles for garment features
KI = INNER // P       # 8 inner tiles

_CACHE = {}


def _build_nc():
    import concourse.bass as bass
    import concourse.tile as tile
    from concourse import bacc, mybir
    from contextlib import ExitStack

    f32 = mybir.dt.float32
    bf16 = mybir.dt.bfloat16
    AF = mybir.ActivationFunctionType
    ALU = mybir.AluOpType

    nc = bacc.Bacc("TRN2", target_bir_lowering=False, debug=False)

    # ---- DRAM parameters (per-core shards; weights replicated) ----
    xp = nc.dram_tensor("xp", [NPC, DQ], bf16, kind="ExternalInput").ap()
    xg = nc.dram_tensor("xg", [M, DC], bf16, kind="ExternalInput").ap()
    wq = nc.dram_tensor("wq", [DQ, INNER], bf16, kind="ExternalInput").ap()
    wk = nc.dram_tensor("wk", [DC, INNER], bf16, kind="ExternalInput").ap()
    wv = nc.dram_tensor("wv", [DC, INNER], bf16, kind="ExternalInput").ap()
    wof = nc.dram_tensor("wof", [INNER, DQ], bf16, kind="ExternalInput").ap()
    wft = nc.dram_tensor("wft", [DQ, DQ], bf16, kind="ExternalInput").ap()
    bq = nc.dram_tensor("bq", [INNER], f32, kind="ExternalInput").ap()
    bk = nc.dram_tensor("bk", [INNER], f32, kind="ExternalInput").ap()
    bv = nc.dram_tensor("bv", [INNER], bf16, kind="ExternalInput").ap()
    bout = nc.dram_tensor("bout", [DQ], f32, kind="ExternalInput").ap()
    out = nc.dram_tensor("out", [NPC, DQ], f32, kind="ExternalOutput").ap()

    # Internal DRAM scratch.
    zp_d = nc.dram_tensor("zp_scratch", [NPC, DQ], bf16).ap()
    zg_d = nc.dram_tensor("zg_scratch", [M, DC], bf16).ap()
    rb_d = nc.dram_tensor("recip_scratch", [H * 2, 512], f32).ap()

    with tile.TileContext(nc) as tc, ExitStack() as ctx:
        psum = ctx.enter_context(tc.tile_pool(name="psum", bufs=8, space="PSUM"))
        const = ctx.enter_context(tc.tile_pool(name="const", bufs=1, side="left"))
        small = ctx.enter_context(tc.tile_pool(name="small", bufs=4, side="left"))

        # ---- constants ----
        eps_t = const.tile([P, 1], f32, name="eps_t")
        nc.vector.memset(eps_t, EPS)
        ones_row = const.tile([1, P], bf16, name="ones_row")
        nc.vector.memset(ones_row, 1.0)
        bq_sb = const.tile([P, KI], f32, name="bq_sb")
        nc.sync.dma_start(out=bq_sb, in_=bq.rearrange("(t p) -> p t", p=P))
        bk_sb = const.tile([P, KI], f32, name="bk_sb")
        nc.sync.dma_start(out=bk_sb, in_=bk.rearrange("(t p) -> p t", p=P))
        bv_row = const.tile([1, INNER], bf16, name="bv_row")
        nc.sync.dma_start(out=bv_row, in_=bv.rearrange("(a d) -> a d", a=1))
        bout_bc = const.tile([P, DQ], f32, name="bout_bc")
        nc.sync.dma_start(
            out=bout_bc,
            in_=bass.AP(tensor=bout.tensor, offset=bout.offset, ap=[[0, P], [1, DQ]]),
        )

        def layernorm_rows(x_t, z_t, d):
            """z = (x - mean(x)) * rsqrt(var(x) + eps), per row of [128, d]."""
            fmax = min(nc.vector.BN_STATS_FMAX, d)
            while d % fmax:
                fmax //= 2
            nsub = d // fmax
            stats = small.tile([P, nsub, nc.vector.BN_STATS_DIM], f32, tag="stats")
            xv = x_t.rearrange("p (s f) -> p s f", s=nsub)
            for s in range(nsub):
                nc.vector.bn_stats(out=stats[:, s, :], in_=xv[:, s, :])
            mv = small.tile([P, nc.vector.BN_AGGR_DIM], f32, tag="mv")
            nc.vector.bn_aggr(out=mv, in_=stats)
            std = small.tile([P, 1], f32, tag="std")
            nc.scalar.activation(out=std, in_=mv[:, 1:2], func=AF.Sqrt, bias=eps_t)
            rstd = small.tile([P, 1], f32, tag="rstd")
            nc.vector.reciprocal(out=rstd, in_=std)
            nc.vector.tensor_scalar(
                out=z_t,
                in0=x_t,
                scalar1=mv[:, 0:1],
                scalar2=rstd,
                op0=ALU.subtract,
                op1=ALU.mult,
            )

        # =========== Phase A-garment: LN + transpose to zgT ===========
        with tc.tile_pool(name="zgt", bufs=KC, side="right") as zgt_pool:
            zgt = [zgt_pool.tile([P, M], bf16, name=f"zgt{j}", tag="zgt") for j in range(KC)]
            with tc.tile_pool(name="gstage", bufs=6, side="right") as gstage:
                for i in range(MT):
                    g_t = gstage.tile([P, DC], bf16, tag="g")
                    nc.sync.dma_start(out=g_t, in_=xg[i * P:(i + 1) * P, :])
                    zg_t = gstage.tile([P, DC], bf16, tag="zg")
                    layernorm_rows(g_t, zg_t, DC)
                    nc.sync.dma_start(out=zg_d[i * P:(i + 1) * P, :], in_=zg_t)
                for j in range(KC):
                    nc.sync.dma_start_transpose(
                        zgt[j], zg_d[:, j * P:(j + 1) * P]
                    )

            # =========== Phase A-person: LN + transpose; xT to DRAM ========
            with tc.tile_pool(name="zpt", bufs=KQ, side="right") as zpt_pool:
                zpt = [zpt_pool.tile([P, NPC], bf16, name=f"zpt{j}", tag="zpt") for j in range(KQ)]
                with tc.tile_pool(name="pstage", bufs=6, side="right") as pstage:
                    for i in range(NT):
                        x_t = pstage.tile([P, DQ], bf16, tag="x")
                        nc.sync.dma_start(out=x_t, in_=xp[i * P:(i + 1) * P, :])
                        z_t = pstage.tile([P, DQ], bf16, tag="z")
                        layernorm_rows(x_t, z_t, DQ)
                        nc.sync.dma_start(out=zp_d[i * P:(i + 1) * P, :], in_=z_t)
                    for j in range(KQ):
                        nc.sync.dma_start_transpose(
                            zpt[j], zp_d[:, j * P:(j + 1) * P]
                        )

                # =========== Phase B-q: qT = Wq'.T @ zpT + bq ===========
                qt_pool = ctx.enter_context(tc.tile_pool(name="qt", bufs=KI, side="left"))
                qt = [qt_pool.tile([P, NPC], bf16, name=f"qt{i}", tag="qt") for i in range(KI)]
                with tc.tile_pool(name="wqk", bufs=3, side="right") as wqk:
                    for it in range(KI):
                        wcol = wqk.tile([P, KQ, P], bf16, tag="w")
                        nc.sync.dma_start(
                            out=wcol,
                            in_=wq[:, it * P:(it + 1) * P].rearrange(
                                "(t p) c -> p t c", p=P
                            ),
                        )
                        for nch in range(NPC // 512):
                            pq = psum.tile([P, 512], f32, tag="ps")
                            for kt in range(KQ):
                                nc.tensor.matmul(
                                    pq,
                                    wcol[:, kt, :],
                                    zpt[kt][:, nch * 512:(nch + 1) * 512],
                                    start=(kt == 0),
                                    stop=(kt == KQ - 1),
                                )
                            nc.scalar.add(
                                out=qt[it][:, nch * 512:(nch + 1) * 512],
                                in_=pq,
                                add=bq_sb[:, it:it + 1],
                            )

            # =========== Phase B-k: kT = Wk'.T @ zgT + bk ===========
            kt_pool = ctx.enter_context(tc.tile_pool(name="kt", bufs=KI, side="left"))
            ktl = [kt_pool.tile([P, M], bf16, name=f"kt{i}", tag="kt") for i in range(KI)]
            with tc.tile_pool(name="wkp", bufs=3, side="right") as wkp:
                for it in range(KI):
                    wcol = wkp.tile([P, KC, P], bf16, tag="w")
                    nc.sync.dma_start(
                        out=wcol,
                        in_=wk[:, it * P:(it + 1) * P].rearrange("(t p) c -> p t c", p=P),
                    )
                    for mch in range(M // 512):
                        pk = psum.tile([P, 512], f32, tag="ps")
                        for kt in range(KC):
                            nc.tensor.matmul(
                                pk,
                                wcol[:, kt, :],
                                zgt[kt][:, mch * 512:(mch + 1) * 512],
                                start=(kt == 0),
                                stop=(kt == KC - 1),
                            )
                        nc.scalar.add(
                            out=ktl[it][:, mch * 512:(mch + 1) * 512],
                            in_=pk,
                            add=bk_sb[:, it:it + 1],
                        )

            # ====== Phase B-v: v[m, inner] = zg @ Wv' + bv, bf16, ones col ==
            v_pool = ctx.enter_context(tc.tile_pool(name="vsb", bufs=MT, side="left"))
            vt = [v_pool.tile([P, H, DH + 1], bf16, name=f"v{i}", tag="v") for i in range(MT)]
            with tc.tile_pool(name="wvp", bufs=12, side="right") as wvp:
                wvt = []
                for kt in range(KC):
                    row = []
                    for ich in range(2):
                        wvc = wvp.tile([P, 512], bf16, tag="wv")
                        nc.sync.dma_start(
                            out=wvc,
                            in_=wv[kt * P:(kt + 1) * P, ich * 512:(ich + 1) * 512],
                        )
                        row.append(wvc)
                    wvt.append(row)
                for mt in range(MT):
                    nc.vector.memset(vt[mt][:, :, DH:DH + 1], 1.0)
                    for ich in range(2):
                        pv = psum.tile([P, 512], f32, tag="ps")
                        for kt in range(KC):
                            nc.tensor.matmul(
                                pv,
                                zgt[kt][:, mt * P:(mt + 1) * P],
                                wvt[kt][ich],
                                start=(kt == 0),
                                stop=False,
                            )
                        nc.tensor.matmul(
                            pv,
                            ones_row,
                            bv_row[:, ich * 512:(ich + 1) * 512],
                            start=False,
                            stop=True,
                        )
                        nc.vector.tensor_copy(
                            vt[mt][:, ich * 8:(ich + 1) * 8, 0:DH],
                            pv.rearrange("p (h d) -> p h d", h=8),
                        )

        # =========== Phase C: attention ===========
        att_pool = ctx.enter_context(tc.tile_pool(name="att", bufs=KI, side="left"))
        att = [att_pool.tile([P, NPC], bf16, name=f"att{i}", tag="att") for i in range(KI)]
        with tc.tile_pool(name="expp", bufs=16, side="right") as expp:
            for h in range(H):
                it_h, row_h = h // 2, (h % 2) * DH
                for nch in range(NPC // 512):
                    pa = psum.tile([P, 512], f32, tag="ps")
                    for mt in range(MT):
                        ps = psum.tile([P, 512], f32, tag="ps")
                        nc.tensor.matmul(
                            ps,
                            ktl[it_h][row_h:row_h + DH, mt * P:(mt + 1) * P],
                            qt[it_h][row_h:row_h + DH, nch * 512:(nch + 1) * 512],
                        )
                        ex = expp.tile([P, 512], bf16, tag="exp")
                        nc.scalar.activation(out=ex, in_=ps, func=AF.Exp)
                        nc.tensor.matmul(
                            pa[0:DH + 1, :],
                            vt[mt][:, h, :],
                            ex,
                            start=(mt == 0),
                            stop=(mt == MT - 1),
                        )
                    idx = h * 2 + nch
                    recip = small.tile([1, 512], f32, tag="recip", bufs=3)
                    nc.vector.reciprocal(out=recip, in_=pa[DH:DH + 1, :])
                    nc.sync.dma_start(out=rb_d[idx:idx + 1, :], in_=recip)
                    bc = small.tile([DH, 512], f32, tag="bc", bufs=3)
                    nc.sync.dma_start(
                        out=bc,
                        in_=bass.AP(
                            tensor=rb_d.tensor, offset=idx * 512, ap=[[0, DH], [1, 512]]
                        ),
                    )
                    nc.vector.tensor_mul(
                        att[it_h][row_h:row_h + DH, nch * 512:(nch + 1) * 512],
                        pa[0:DH, :],
                        bc,
                    )

        # =========== Phase D: out = xT.T @ Wf_top + attT.T @ WoF + bias ====
        with (
            tc.tile_pool(name="wofp", bufs=16, side="right") as wofp,
            tc.tile_pool(name="wftp", bufs=KQ, side="right") as wftp,
            tc.tile_pool(name="xptr", bufs=KQ, side="right") as xptrp,
            tc.tile_pool(name="outp", bufs=4, side="right") as outp,
        ):
            woft = []
            for it in range(KI):
                row = []
                for ch in range(2):
                    wo_t = wofp.tile([P, 512], bf16, tag="wof")
                    nc.sync.dma_start(
                        out=wo_t,
                        in_=wof[it * P:(it + 1) * P, ch * 512:(ch + 1) * 512],
                    )
                    row.append(wo_t)
                woft.append(row)
            xptr = []
            for kt in range(KQ):
                xr = xptrp.tile([P, NPC], bf16, tag="xpt")
                nc.sync.dma_start_transpose(xr, xp[:, kt * P:(kt + 1) * P])
                xptr.append(xr)
            for ch in range(2):
                wftt = []
                for kt in range(KQ):
                    wf_t = wftp.tile([P, 512], bf16, tag="wft")
                    nc.sync.dma_start(
                        out=wf_t,
                        in_=wft[kt * P:(kt + 1) * P, ch * 512:(ch + 1) * 512],
                    )
                    wftt.append(wf_t)
                for nt in range(NT):
                    pf = psum.tile([P, 512], f32, tag="ps")
                    for kt in range(KQ):
                        nc.tensor.matmul(
                            pf,
                            xptr[kt][:, nt * P:(nt + 1) * P],
                            wftt[kt],
                            start=(kt == 0),
                            stop=False,
                        )
                    for it in range(KI):
                        nc.tensor.matmul(
                            pf,
                            att[it][:, nt * P:(nt + 1) * P],
                            woft[it][ch],
                            start=False,
                            stop=(it == KI - 1),
                        )
                    o_t = outp.tile([P, 512], f32, tag="o")
                    nc.vector.tensor_add(o_t, pf, bout_bc[:, ch * 512:(ch + 1) * 512])
                    nc.sync.dma_start(
                        out=out[nt * P:(nt + 1) * P, ch * 512:(ch + 1) * 512],
                        in_=o_t,
                    )

    nc.compile()
    return nc


def get_nc():
    if "nc" not in _CACHE:
        _CACHE["nc"] = _build_nc()
    return _CACHE["nc"]


def make_in_maps(inputs):
    """Host-side folding + sharding. Returns one input dict per core."""
    bf = ml_dtypes.bfloat16
    pf_ = np.asarray(inputs["person_features"], np.float32)
    gf_ = np.asarray(inputs["garment_features"], np.float32)
    Wq = np.asarray(inputs["Wq"], np.float32)
    Wk = np.asarray(inputs["Wk"], np.float32)
    Wv = np.asarray(inputs["Wv"], np.float32)
    Wo = np.asarray(inputs["Wo"], np.float32)
    bo = np.asarray(inputs["bo"], np.float32)
    Wf = np.asarray(inputs["Wf"], np.float32)
    bff = np.asarray(inputs["bf"], np.float32)
    gq = np.asarray(inputs["gq"], np.float32)
    betaq = np.asarray(inputs["betaq"], np.float32)
    gk = np.asarray(inputs["gk"], np.float32)
    betak = np.asarray(inputs["betak"], np.float32)

    wq_f = (gq[:, None] * Wq) * np.float32(SCALE)
    bq_f = (betaq @ Wq) * np.float32(SCALE)
    wk_f = gk[:, None] * Wk
    bk_f = betak @ Wk
    wv_f = gk[:, None] * Wv
    bv_f = betak @ Wv
    wf_top = np.ascontiguousarray(Wf[:DQ])
    wf_bot = Wf[DQ:]
    wof = (Wo.astype(np.float64) @ wf_bot.astype(np.float64)).astype(np.float32)
    bout = (bo @ wf_bot + bff).astype(np.float32)

    shared = {
        "wq": np.ascontiguousarray(wq_f).astype(bf),
        "wk": np.ascontiguousarray(wk_f).astype(bf),
        "wv": np.ascontiguousarray(wv_f).astype(bf),
        "wof": wof.astype(bf),
        "wft": wf_top.astype(bf),
        "bq": np.ascontiguousarray(bq_f),
        "bk": np.ascontiguousarray(bk_f),
        "bv": np.ascontiguousarray(bv_f).astype(bf),
        "bout": bout,
    }
    in_maps = []
    for core in range(NCORES):
        b, half = divmod(core, 2)
        m = dict(shared)
        m["xp"] = np.ascontiguousarray(pf_[b, half * NPC:(half + 1) * NPC]).astype(bf)
        m["xg"] = np.ascontiguousarray(gf_[b]).astype(bf)
        in_maps.append(m)
    return in_maps


def assemble(results):
    out = np.empty((B, N, DQ), np.float32)
    for core in range(NCORES):
        b, half = divmod(core, 2)
        out[b, half * NPC:(half + 1) * NPC] = results[core]["out"]
    return out


def kernel(**inputs):
    from concourse.bass_utils import run_bass_kernel_spmd

    nc = get_nc()
    in_maps = make_in_maps(inputs)
    res = run_bass_kernel_spmd(nc, in_maps, list(range(NCORES)))
    return assemble(res.results)

